# revision 16
# baseline (speedup 1.0000x reference)
"""Trainium2 Bass kernel for PiecewiseHawkesIntensity.

Per (b, p, query q):
  qn = q / norm_b                       (fp32, reference rounding semantics)
  j  = clip(searchsorted(events, qn, left) - 1, 0)
  out[b, m, p, q] = (mu[j] + (alpha[j]-mu[j]) * exp(-beta[j]*(qn - t_j))) / norm_b

Device strategy (per core, 2 batch elements, data-parallel over B):
  The searchsorted + gather is done WITHOUT indirect DMA:

  1. Exact thresholds: T_j = pred_fp32((t_j + ulp(t_j)/2) * nrm) computed with
     a Dekker two-product so that the fp32 compare  [q > T_j]  reproduces the
     reference's  [t_j < RN(q/nrm)]  bit-exactly (up to measure-zero ties).
  2. S_j = [q > T_j] and S_{j+1} via a shifted threshold column; the one-hot
     oh_j = S_j - S_{j+1} (row 0 = [count<=1]) selects exactly j(q).
  3. Gather = PE matmul: G[v, q] = sum_j VT[j, v] * oh[j, q] in fp32r.  Each
     output is a single f32r-rounded product (one-hot), so values keep ~2^-13
     relative accuracy; t is carried as an exact 3-way bf16 split (A+B+C) in
     three lhsT columns to preserve fp32-level accuracy of delta_t.
  4. Epilogue on DVE/ACT: dt = qn - t_last*mask, out = mu' + (al'-mu')*exp(-b dt)
     with 1/nrm folded into the PSUM->SBUF copies.
"""

import sys

sys.path.insert(0, "/opt/trn_rl_repo")

from contextlib import ExitStack

import numpy as np

import concourse.bass as bass
import concourse.bacc as bacc
import concourse.mybir as mybir
import concourse.tile as tile
from concourse import masks
from concourse.bass_types import AP

F32 = mybir.dt.float32
F32R = mybir.dt.float32r
F16 = mybir.dt.float16
BF16 = mybir.dt.bfloat16
I32 = mybir.dt.int32
OP = mybir.AluOpType
AFT = mybir.ActivationFunctionType

B, M, P, L, LE = 16, 32, 16, 1024, 2048
NB = 2
NCORES = 8
NV = 99          # value rows: 32 mu | 32 al | 32 be | tA | tB | tC
BIG = 1.0e30
LN2 = 0.6931471805599453
SPLIT = 4097.0   # Veltkamp constant for fp32 (2^12 + 1)


def _bcast(ap_col, n):
    """[P, 1] column -> [P, n] via free-stride-0."""
    return AP(ap_col.tensor, ap_col.offset, [ap_col.ap[0], [0, n]])


def build_program(nc: bass.Bass):
    qt_h = nc.declare_dram_parameter("qt", [NB, P, LE + L], F32, isOutput=False)
    vals_h = nc.declare_dram_parameter("vals", [NB, 3, M, P, L], F16, isOutput=False)
    nrm_h = nc.declare_dram_parameter("nrm", [NB, 128, 1], F32, isOutput=False)
    out_h = nc.declare_dram_parameter("out", [NB, M, P, LE], F16, isOutput=True)

    with tile.TileContext(nc) as tc, ExitStack() as ctx:
        const = ctx.enter_context(tc.tile_pool(name="const", bufs=1))
        perb = ctx.enter_context(tc.tile_pool(name="perb", bufs=2))
        sml = ctx.enter_context(tc.tile_pool(name="sml", bufs=2))
        cmp = ctx.enter_context(tc.tile_pool(name="cmp", bufs=2))
        vpool = ctx.enter_context(tc.tile_pool(name="vp", bufs=2))
        epi = ctx.enter_context(tc.tile_pool(name="epi", bufs=1))
        psg = ctx.enter_context(tc.tile_pool(name="psg", bufs=1, space="PSUM"))
        pst = ctx.enter_context(tc.tile_pool(name="pst", bufs=2, space="PSUM"))

        ident = const.tile([128, 128], F32)
        masks.make_identity(nc, ident[:])
        expb = const.tile([128, 1], F32)
        nc.vector.memset(expb[:], -23.0 * LN2)

        for b in range(NB):
            # ---- 1/nrm (Newton-refined reciprocal) ----
            nrm_t = perb.tile([128, 1], F32, tag="nrm")
            nc.sync.dma_start(nrm_t[:], nrm_h[b])
            r0 = perb.tile([128, 1], F32, tag="r0")
            nc.vector.reciprocal(r0[:], nrm_t[:])
            er = perb.tile([128, 1], F32, tag="er")
            nc.vector.tensor_tensor(er[:], nrm_t[:], r0[:], op=OP.mult)
            nc.vector.tensor_scalar(er[:], er[:], -1.0, 2.0, op0=OP.mult, op1=OP.add)
            rr = perb.tile([128, 1], F32, tag="rr")
            nc.vector.tensor_tensor(rr[:], r0[:], er[:], op=OP.mult)

            # ---- events in [128, 128] layout: row pi = p*8+jc, col f = j%128 ----
            tt = perb.tile([128, 128], F32, tag="tt")
            ta_src = qt_h[b]
            nc.sync.dma_start(
                tt[:],
                AP(ta_src.tensor, ta_src.offset + LE,
                   [[LE + L, P], [128, 8], [1, 128]]),
            )

            # ---- Veltkamp split of t and nrm ----
            ct = perb.tile([128, 128], F32, tag="ct")
            nc.vector.tensor_scalar(ct[:], tt[:], SPLIT, None, op0=OP.mult)
            th = perb.tile([128, 128], F32, tag="th")
            nc.vector.tensor_tensor(th[:], ct[:], tt[:], op=OP.subtract)  # c - t
            nc.vector.tensor_tensor(th[:], ct[:], th[:], op=OP.subtract)  # t_hi
            tl = perb.tile([128, 128], F32, tag="tl")
            nc.vector.tensor_tensor(tl[:], tt[:], th[:], op=OP.subtract)  # t_lo

            cn = perb.tile([128, 1], F32, tag="cn")
            nc.vector.tensor_scalar(cn[:], nrm_t[:], SPLIT, None, op0=OP.mult)
            nh = perb.tile([128, 1], F32, tag="nh")
            nc.vector.tensor_tensor(nh[:], cn[:], nrm_t[:], op=OP.subtract)
            nc.vector.tensor_tensor(nh[:], cn[:], nh[:], op=OP.subtract)
            nl = perb.tile([128, 1], F32, tag="nl")
            nc.vector.tensor_tensor(nl[:], nrm_t[:], nh[:], op=OP.subtract)

            # ---- P = RN(t*nrm), e1 = exact residual (Dekker) ----
            Pt = perb.tile([128, 128], F32, tag="Pt")
            nc.vector.tensor_scalar(Pt[:], tt[:], nrm_t[:], None, op0=OP.mult)
            e1 = perb.tile([128, 128], F32, tag="e1")
            nc.vector.tensor_scalar(e1[:], th[:], nh[:], None, op0=OP.mult)
            nc.vector.tensor_tensor(e1[:], e1[:], Pt[:], op=OP.subtract)
            x2 = perb.tile([128, 128], F32, tag="x2")
            nc.vector.tensor_scalar(x2[:], th[:], nl[:], None, op0=OP.mult)
            nc.vector.tensor_tensor(e1[:], e1[:], x2[:], op=OP.add)
            nc.vector.tensor_scalar(x2[:], tl[:], nh[:], None, op0=OP.mult)
            nc.vector.tensor_tensor(e1[:], e1[:], x2[:], op=OP.add)
            nc.vector.tensor_scalar(x2[:], tl[:], nl[:], None, op0=OP.mult)
            nc.vector.tensor_tensor(e1[:], e1[:], x2[:], op=OP.add)

            # ---- ulp(x) = 2^(floor(log2 x) - 23) ----
            def make_ulp(src, outtag):
                lg = perb.tile([128, 128], F32, tag="ulg")
                nc.scalar.activation(lg[:], src[:], AFT.Ln)
                nc.vector.tensor_scalar(lg[:], lg[:], 1.0 / LN2, None, op0=OP.mult)
                lgi = perb.tile([128, 128], I32, tag="ulgi")
                nc.vector.tensor_copy(lgi[:], lg[:])
                lgf = perb.tile([128, 128], F32, tag="ulgf")
                nc.vector.tensor_copy(lgf[:], lgi[:])
                fx = perb.tile([128, 128], F32, tag="ufx")
                nc.vector.tensor_tensor(fx[:], lgf[:], lg[:], op=OP.is_gt)
                nc.vector.tensor_tensor(lgf[:], lgf[:], fx[:], op=OP.subtract)
                ul = perb.tile([128, 128], F32, tag=outtag)
                nc.scalar.activation(ul[:], lgf[:], AFT.Exp, scale=LN2, bias=expb[:])
                return ul

            ulpt = make_ulp(tt, "ulpt")
            # c = ulp(t) * nrm * 0.5 ; delta = e1 + c ; w = 0.75*(1 - [delta>0])
            cterm = perb.tile([128, 128], F32, tag="cterm")
            nc.vector.tensor_scalar(cterm[:], ulpt[:], nrm_t[:], 0.5, op0=OP.mult, op1=OP.mult)
            nc.vector.tensor_tensor(cterm[:], e1[:], cterm[:], op=OP.add)
            sel = perb.tile([128, 128], F32, tag="sel")
            nc.vector.tensor_scalar(sel[:], cterm[:], 0.0, -1.0, op0=OP.is_gt, op1=OP.mult)
            nc.vector.tensor_scalar(sel[:], sel[:], 1.0, 0.75, op0=OP.add, op1=OP.mult)
            ulpP = make_ulp(Pt, "ulpP")
            nc.vector.tensor_tensor(ulpP[:], ulpP[:], sel[:], op=OP.mult)

            # ---- T = P - w*ulp(P)  in [128, 128] layout ----
            T128 = perb.tile([128, 128], F32, tag="T128")
            nc.vector.tensor_tensor(T128[:], Pt[:], ulpP[:], op=OP.subtract)
            # shifted (flat j+1) version: cols 0..126 from col+1, col 127 from
            # next partition's col 0 (BIG for the last chunk of each path)
            T128s = perb.tile([128, 128], F32, tag="T128s")
            nc.vector.tensor_copy(T128s[:, 0:127], T128[:, 1:128])
            nc.vector.memset(T128s[:, 127:128], BIG)
            for p in range(P):
                nc.sync.dma_start(T128s[8 * p:8 * p + 7, 127:128],
                                  T128[8 * p + 1:8 * p + 8, 0:1])

            # ---- transpose thresholds: TT[j%128, p*8+jc] ----
            TT = perb.tile([128, 128], F32, tag="TT")
            psT = pst.tile([128, 128], F32, tag="ps1")
            nc.tensor.matmul(psT[:], T128[:], ident[:], is_transpose=True)
            nc.scalar.activation(TT[:], psT[:], AFT.Copy)
            TTs = perb.tile([128, 128], F32, tag="TTs")
            psT2 = pst.tile([128, 128], F32, tag="ps1")
            nc.tensor.matmul(psT2[:], T128s[:], ident[:], is_transpose=True)
            nc.scalar.activation(TTs[:], psT2[:], AFT.Copy)

            # ---- exact 3-way bf16 split of t:  t = A + B + C ----
            Abf = perb.tile([128, 128], BF16, tag="Abf")
            nc.vector.tensor_copy(Abf[:], tt[:])
            Af = perb.tile([128, 128], F32, tag="Af")
            nc.vector.tensor_copy(Af[:], Abf[:])
            R1 = perb.tile([128, 128], F32, tag="R1")
            nc.vector.tensor_tensor(R1[:], tt[:], Af[:], op=OP.subtract)
            Bbf = perb.tile([128, 128], BF16, tag="Bbf")
            nc.vector.tensor_copy(Bbf[:], R1[:])
            Bf = perb.tile([128, 128], F32, tag="Bf")
            nc.vector.tensor_copy(Bf[:], Bbf[:])
            Cf = perb.tile([128, 128], F32, tag="Cf")
            nc.vector.tensor_tensor(Cf[:], R1[:], Bf[:], op=OP.subtract)

            for p in range(P):
                # ---- replicate raw q across 128 partitions ----
                qrep = sml.tile([128, LE], F32, tag="qrep")
                qa = qt_h[b, p, 0:LE]
                nc.sync.dma_start(qrep[:], AP(qa.tensor, qa.offset, [[0, 128], [1, LE]]))

                # ---- V = [mu; al; be; A; B; C] (raw values) ----
                V = vpool.tile([NV, L], F32, tag="V")
                V16 = vpool.tile([96, L], F16, tag="V16")
                nc.sync.dma_start(V16[0:32, :], vals_h[b, 0, :, p, :])
                nc.sync.dma_start(V16[32:64, :], vals_h[b, 1, :, p, :])
                nc.sync.dma_start(V16[64:96, :], vals_h[b, 2, :, p, :])
                nc.vector.tensor_copy(V[0:96, :], V16[0:96, :])
                nc.sync.dma_start(V[96:97, :], Af[p * 8:(p + 1) * 8, :])
                nc.sync.dma_start(V[97:98, :], Bf[p * 8:(p + 1) * 8, :])
                nc.sync.dma_start(V[98:99, :], Cf[p * 8:(p + 1) * 8, :])

                VT = vpool.tile([128, NV * 8], F32R, tag="VT")
                for jc in range(8):
                    psv = pst.tile([128, NV], F32, tag="ps1")
                    nc.tensor.matmul(psv[:], V[:, jc * 128:(jc + 1) * 128],
                                     ident[0:NV, 0:NV], is_transpose=True)
                    nc.scalar.activation(VT[:, jc * NV:(jc + 1) * NV], psv[:], AFT.Copy)

                # ---- compares + one-hot + gather matmuls ----
                G = psg.tile([NV, LE], F32, tag="G")
                mask_f = epi.tile([1, LE], F32, tag="mask")
                for c in range(8):
                    Sc = cmp.tile([128, LE], BF16, tag="Sc")
                    nc.vector.tensor_tensor(
                        Sc[:], qrep[:], _bcast(TT[:, p * 8 + c:p * 8 + c + 1], LE),
                        op=OP.is_gt)
                    S2c = cmp.tile([128, LE], BF16, tag="S2c")
                    nc.vector.tensor_tensor(
                        S2c[:], qrep[:], _bcast(TTs[:, p * 8 + c:p * 8 + c + 1], LE),
                        op=OP.is_gt)
                    oh = cmp.tile([128, LE], F32R, tag="oh")
                    nc.vector.tensor_tensor(oh[:], Sc[:], S2c[:], op=OP.subtract)
                    if c == 0:
                        nc.vector.tensor_scalar(oh[0:1, :], S2c[0:1, :], -1.0, 1.0,
                                                op0=OP.mult, op1=OP.add)
                        nc.vector.tensor_copy(mask_f[:], Sc[0:1, :])
                    for f in range(4):
                        nc.tensor.matmul(
                            G[:, f * 512:(f + 1) * 512],
                            VT[:, c * NV:(c + 1) * NV],
                            oh[:, f * 512:(f + 1) * 512],
                            start=(c == 0), stop=(c == 7))

                # ---- epilogue ----
                Gmu = epi.tile([32, LE], F32, tag="Gmu")
                nc.scalar.activation(Gmu[:], G[0:32, :], AFT.Copy, scale=rr[0:32])
                Gal = epi.tile([32, LE], F32, tag="Gal")
                nc.scalar.activation(Gal[:], G[32:64, :], AFT.Copy, scale=rr[32:64])
                Gbe = epi.tile([32, LE], F32, tag="Gbe")
                nc.scalar.activation(Gbe[:], G[64:96, :], AFT.Copy, scale=-1.0)
                Gt = epi.tile([3, LE], F32, tag="Gt")
                nc.scalar.activation(Gt[:], G[96:99, :], AFT.Copy)
                Gt1 = epi.tile([1, LE], F32, tag="Gt1")
                nc.sync.dma_start(Gt1[:], Gt[1:2, :])
                Gt2 = epi.tile([1, LE], F32, tag="Gt2")
                nc.sync.dma_start(Gt2[:], Gt[2:3, :])

                tsum = epi.tile([1, LE], F32, tag="tsum")
                nc.vector.tensor_tensor(tsum[:], Gt[0:1, :], Gt1[:], op=OP.add)
                nc.vector.tensor_tensor(tsum[:], tsum[:], Gt2[:], op=OP.add)
                nc.vector.tensor_tensor(tsum[:], tsum[:], mask_f[:], op=OP.mult)

                qrow = epi.tile([1, LE], F32, tag="qrow")
                nc.sync.dma_start(qrow[:], qt_h[b, p, 0:LE])
                dtt = epi.tile([32, LE], F32, tag="dtt")
                nc.vector.tensor_scalar(qrow[:], qrow[:], rr[0:1], None, op0=OP.mult)
                nc.vector.tensor_tensor(dtt[0:1, :], qrow[:], tsum[:], op=OP.subtract)
                dtrep = epi.tile([32, LE], F32, tag="dtrep")
                nc.vector.stream_shuffle(dtrep[:], dtt[:], mask=[0] * 32)

                u = epi.tile([32, LE], F32, tag="u")
                nc.vector.tensor_tensor(u[:], Gbe[:], dtrep[:], op=OP.mult)
                E = epi.tile([32, LE], F32, tag="E")
                nc.scalar.activation(E[:], u[:], AFT.Exp)
                O = epi.tile([32, LE], F32, tag="O")
                nc.vector.tensor_tensor(O[:], Gal[:], Gmu[:], op=OP.subtract)
                nc.vector.tensor_tensor(O[:], O[:], E[:], op=OP.mult)
                O16 = epi.tile([32, LE], F16, tag="O16")
                nc.vector.tensor_tensor(O16[:], O[:], Gmu[:], op=OP.add)
                nc.sync.dma_start(out_h[b, :, p, :], O16[:])

    nc.compile()
    return nc


def _host_globals(query_times, event_times, mu, alpha, beta, norm_constants):
    """Global (all-cores) input arrays, already in the sharded layout: axis 0
    is the batch dim that shard_map splits 8 ways (2 batches per core)."""
    q = np.ascontiguousarray(query_times, dtype=np.float32)
    t = np.ascontiguousarray(event_times, dtype=np.float32)
    nrm = np.ascontiguousarray(norm_constants, dtype=np.float32)
    qt = np.concatenate([q, t], axis=2)                    # [B, P, LE+L] f32
    vals = np.empty((B, 3, M, P, L), np.float16)           # [B, 3, M, P, L]
    vals[:, 0] = mu
    vals[:, 1] = alpha
    vals[:, 2] = beta
    nrm_rep = np.repeat(nrm.reshape(B, 1, 1), 128, axis=1).astype(np.float32)
    return {"qt": qt, "vals": vals, "nrm": nrm_rep}


def _per_core_maps(g):
    return [
        {k: np.ascontiguousarray(v[NB * i:NB * (i + 1)]) for k, v in g.items()}
        for i in range(NCORES)
    ]


_NC_CACHE = {}
_SETUP: dict = {}


def _aot_cache_path():
    import hashlib
    h = hashlib.sha256(open(__file__, "rb").read()).hexdigest()[:16]
    import os
    d = os.path.expanduser("~/.neuron-compile-cache")
    os.makedirs(d, exist_ok=True)
    return os.path.join(d, f"hawkes_aot_{h}.pkl")


def _try_load_aot():
    """Load serialized AOT executables from disk; skips build + compile."""
    import pickle
    import jax
    from jax.experimental.serialize_executable import deserialize_and_load
    from jax.sharding import Mesh, NamedSharding, PartitionSpec

    with open(_aot_cache_path(), "rb") as f:
        blob = pickle.load(f)
    devices = jax.devices()[:NCORES]
    assert len(devices) == NCORES
    mesh = Mesh(np.asarray(devices), ("core",))
    ns = NamedSharding(mesh, PartitionSpec("core"))
    compiled = deserialize_and_load(*blob["compiled"])
    zero_fn = deserialize_and_load(*blob["zeros"])
    _SETUP.update(
        groups=[dict(ns=ns, compiled=compiled, zero_fn=zero_fn, ncore=NCORES)],
        in_names=blob["in_names"], out_names=blob["out_names"], jax=jax,
    )
    return _SETUP


def _save_aot():
    import pickle
    from jax.experimental.serialize_executable import serialize

    g = _SETUP["groups"][0]
    blob = {
        "compiled": serialize(g["compiled"]),
        "zeros": serialize(g["zero_fn"]),
        "in_names": _SETUP["in_names"],
        "out_names": _SETUP["out_names"],
    }
    tmp = _aot_cache_path() + ".tmp"
    with open(tmp, "wb") as f:
        pickle.dump(blob, f)
    import os
    os.replace(tmp, _aot_cache_path())


def _ensure_setup():
    """One-time: build the Bass program, AOT-compile the sharded PJRT
    executable (NEFF comes from the persistent neuron cache when warm), and
    prepare a device-side zero-buffer maker for the donated outputs.  Doing
    this at import keeps the kernel() call itself transfer-bound."""
    if "groups" in _SETUP:
        return _SETUP
    try:
        return _try_load_aot()
    except Exception:
        pass
    import jax
    import jax.numpy as jnp
    from jax.experimental.shard_map import shard_map
    from jax.sharding import Mesh, NamedSharding, PartitionSpec
    from concourse import bass2jax as B2J

    nc = _NC_CACHE.get("nc")
    if nc is None:
        nc = _NC_CACHE["nc"] = build_program(bacc.Bacc())
    B2J.install_neuronx_cc_hook()

    partition_name = nc.partition_id_tensor.name if nc.partition_id_tensor else None
    in_names, in_avals, out_names, out_avals = [], [], [], []
    for alloc in nc.m.functions[0].allocations:
        if not isinstance(alloc, mybir.MemoryLocationSet):
            continue
        name = alloc.memorylocations[0].name
        if alloc.kind == "ExternalInput":
            if name != partition_name:
                in_names.append(name)
                in_avals.append((tuple(alloc.tensor_shape), mybir.dt.np(alloc.dtype)))
        elif alloc.kind == "ExternalOutput":
            out_names.append(name)
            out_avals.append(jax.core.ShapedArray(tuple(alloc.tensor_shape),
                                                  mybir.dt.np(alloc.dtype)))
    n_params, n_outs = len(in_names), len(out_avals)
    all_names = list(in_names) + list(out_names)
    if partition_name is not None:
        all_names.append(partition_name)
    donate = tuple(range(n_params, n_params + n_outs))

    def _body(*args):
        operands = list(args)
        if partition_name is not None:
            operands.append(B2J.partition_id_tensor())
        outs = B2J._bass_exec_p.bind(
            *operands,
            out_avals=tuple(out_avals),
            in_names=tuple(all_names),
            out_names=tuple(out_names),
            lowering_input_output_aliases=(),
            sim_require_finite=True,
            sim_require_nnan=True,
            nc=nc,
        )
        return tuple(outs)

    all_devices = jax.devices()[:NCORES]
    assert len(all_devices) == NCORES
    spec = PartitionSpec("core")

    def make_group(devices):
        ncore = len(devices)
        mesh = Mesh(np.asarray(devices), ("core",))
        ns = NamedSharding(mesh, spec)
        sharded = jax.jit(
            shard_map(_body, mesh=mesh, in_specs=(spec,) * (n_params + n_outs),
                      out_specs=(spec,) * n_outs, check_rep=False),
            donate_argnums=donate, keep_unused=True,
        )
        in_structs = [
            jax.ShapeDtypeStruct((ncore * s[0], *s[1:]), d, sharding=ns)
            for (s, d) in in_avals
        ]
        out_structs = [
            jax.ShapeDtypeStruct((ncore * a.shape[0], *a.shape[1:]), a.dtype,
                                 sharding=ns)
            for a in out_avals
        ]
        compiled = sharded.lower(*in_structs, *out_structs).compile()
        zero_specs = [((ncore * a.shape[0], *a.shape[1:]), a.dtype)
                      for a in out_avals]
        zero_fn = jax.jit(
            lambda: tuple(jnp.zeros(s, d) for (s, d) in zero_specs),
            out_shardings=(ns,) * n_outs,
        ).lower().compile()
        return dict(ns=ns, compiled=compiled, zero_fn=zero_fn, ncore=ncore)

    # Single full-machine group: split-pipelining measured slower on the
    # axon relay (head-of-line blocking between interleaved up/down streams).
    groups = [make_group(all_devices)]

    _SETUP.update(
        groups=groups, in_names=in_names, out_names=out_names,
        out_avals=out_avals, jax=jax,
    )
    try:
        _save_aot()
    except Exception:
        pass
    return _SETUP


def _put_sharded(jax, arr, ns):
    """Per-shard async device_put + global assembly (overlaps host work of
    later shards with relay transfer of earlier ones)."""
    devs = list(ns.mesh.devices.flat)
    n = len(devs)
    step = arr.shape[0] // n
    shards = [
        jax.device_put(arr[i * step:(i + 1) * step], devs[i]) for i in range(n)
    ]
    return jax.make_array_from_single_device_arrays(arr.shape, ns, shards)


def _run_fast(query_times, event_times, mu, alpha, beta, norm_constants):
    s = _ensure_setup()
    jax = s["jax"]
    g = s["groups"][0]
    ns = g["ns"]
    devs = list(ns.mesh.devices.flat)

    q = np.ascontiguousarray(query_times, dtype=np.float32)
    t = np.ascontiguousarray(event_times, dtype=np.float32)
    nrm = np.ascontiguousarray(norm_constants, dtype=np.float32)
    qt = np.concatenate([q, t], axis=2)
    nrm_rep = np.repeat(nrm.reshape(B, 1, 1), 128, axis=1).astype(np.float32)

    # vals: cast each core's shard to fp16 then queue its upload immediately,
    # so the next shard's cast overlaps the previous shard's relay transfer.
    vals_shards = []
    for i in range(NCORES):
        sl = slice(NB * i, NB * (i + 1))
        ch = np.empty((NB, 3, M, P, L), np.float16)
        ch[:, 0] = mu[sl]
        ch[:, 1] = alpha[sl]
        ch[:, 2] = beta[sl]
        vals_shards.append(jax.device_put(ch, devs[i]))
    vals_g = jax.make_array_from_single_device_arrays(
        (B, 3, M, P, L), ns, vals_shards)
    by_name = {
        "qt": _put_sharded(jax, qt, ns),
        "vals": vals_g,
        "nrm": _put_sharded(jax, nrm_rep, ns),
    }
    dev_in = [by_name[name] for name in s["in_names"]]
    zeros = g["zero_fn"]()
    outs = g["compiled"](*dev_in, *zeros)

    out_g = outs[s["out_names"].index("out")]
    res = np.empty((B, M, P, LE), np.float32)
    import concurrent.futures as cf

    def fetch(sh):
        res[sh.index] = np.asarray(sh.data)  # 4MB fetch + fp16->f32 cast

    with cf.ThreadPoolExecutor(max_workers=NCORES) as ex:
        list(ex.map(fetch, out_g.addressable_shards))
    return res


def _numpy_fallback(query_times, event_times, mu, alpha, beta, norm_constants):
    q_norm = (query_times / norm_constants[:, None, None]).astype(np.float32)
    Bq, Pq, Le = q_norm.shape
    last = np.empty((Bq, Pq, Le), np.int64)
    for b in range(Bq):
        for p in range(Pq):
            last[b, p] = np.searchsorted(event_times[b, p], q_norm[b, p], "left") - 1
    idx = np.clip(last, 0, None)
    g = np.broadcast_to(idx[:, None], (Bq, mu.shape[1], Pq, Le))
    mu_l = np.take_along_axis(mu, g, 3)
    al_l = np.take_along_axis(alpha, g, 3)
    be_l = np.take_along_axis(beta, g, 3)
    t_l = np.where(last == -1, 0.0, np.take_along_axis(event_times, idx, 2))
    dt = (q_norm - t_l)[:, None]
    out = mu_l + (al_l - mu_l) * np.exp(-be_l * dt)
    return (out / norm_constants[:, None, None, None]).astype(np.float32)


def kernel(query_times, event_times, mu, alpha, beta, norm_constants):
    try:
        return _run_fast(query_times, event_times, mu, alpha, beta,
                         norm_constants)
    except Exception as e:
        print(f"WARNING: fast path failed ({e!r}); trying run_bass_kernel_spmd",
              file=sys.stderr)
    g = _host_globals(query_times, event_times, mu, alpha, beta, norm_constants)
    try:
        from concourse.bass_utils import run_bass_kernel_spmd

        if "nc" not in _NC_CACHE:
            _NC_CACHE["nc"] = build_program(bacc.Bacc())
        nc = _NC_CACHE["nc"]
        res = run_bass_kernel_spmd(nc, _per_core_maps(g), core_ids=list(range(NCORES)))
        outs = [res.results[i]["out"].astype(np.float32) for i in range(NCORES)]
        return np.concatenate(outs, axis=0)
    except Exception as e:  # pragma: no cover
        print(
            "WARNING: device execution failed; returning HOST numpy fallback "
            f"(no hardware time was measured). Device error: {e!r}",
            file=sys.stderr,
        )
        return _numpy_fallback(
            np.asarray(query_times, np.float32),
            np.asarray(event_times, np.float32),
            np.asarray(mu, np.float32),
            np.asarray(alpha, np.float32),
            np.asarray(beta, np.float32),
            np.asarray(norm_constants, np.float32),
        )


# Import-time warmup: build + AOT compile so the kernel() call itself is
# transfer-bound.  Best-effort — kernel() retries lazily on failure.
try:
    _ensure_setup()
except Exception:
    pass


# revision 17
# speedup vs baseline: 1.3688x; 1.3688x over previous
"""Trainium2 Bass kernel for PiecewiseHawkesIntensity.

Per (b, p, query q):
  qn = q / norm_b                       (fp32, reference rounding semantics)
  j  = clip(searchsorted(events, qn, left) - 1, 0)
  out[b, m, p, q] = (mu[j] + (alpha[j]-mu[j]) * exp(-beta[j]*(qn - t_j))) / norm_b

Device strategy (per core, 2 batch elements, data-parallel over B):
  The searchsorted + gather is done WITHOUT indirect DMA:

  1. Exact thresholds: T_j = pred_fp32((t_j + ulp(t_j)/2) * nrm) computed with
     a Dekker two-product so that the fp32 compare  [q > T_j]  reproduces the
     reference's  [t_j < RN(q/nrm)]  bit-exactly (up to measure-zero ties).
  2. S_j = [q > T_j] and S_{j+1} via a shifted threshold column; the one-hot
     oh_j = S_j - S_{j+1} (row 0 = [count<=1]) selects exactly j(q).
  3. Gather = PE matmul: G[v, q] = sum_j VT[j, v] * oh[j, q] in fp32r.  Each
     output is a single f32r-rounded product (one-hot), so values keep ~2^-13
     relative accuracy; t is carried as an exact 3-way bf16 split (A+B+C) in
     three lhsT columns to preserve fp32-level accuracy of delta_t.
  4. Epilogue on DVE/ACT: dt = qn - t_last*mask, out = mu' + (al'-mu')*exp(-b dt)
     with 1/nrm folded into the PSUM->SBUF copies.
"""

import sys

sys.path.insert(0, "/opt/trn_rl_repo")

from contextlib import ExitStack

import numpy as np

import concourse.bass as bass
import concourse.bacc as bacc
import concourse.mybir as mybir
import concourse.tile as tile
from concourse import masks
from concourse.bass_types import AP

F32 = mybir.dt.float32
F32R = mybir.dt.float32r
F16 = mybir.dt.float16
BF16 = mybir.dt.bfloat16
I32 = mybir.dt.int32
OP = mybir.AluOpType
AFT = mybir.ActivationFunctionType

B, M, P, L, LE = 16, 32, 16, 1024, 2048
QTW = LE + L + 8      # q | t | nrm-pad columns
NB = 2
NCORES = 8
NV = 99          # value rows: 32 mu | 32 al | 32 be | tA | tB | tC
BIG = 1.0e30
LN2 = 0.6931471805599453
SPLIT = 4097.0   # Veltkamp constant for fp32 (2^12 + 1)


def _bcast(ap_col, n):
    """[P, 1] column -> [P, n] via free-stride-0."""
    return AP(ap_col.tensor, ap_col.offset, [ap_col.ap[0], [0, n]])


def build_program(nc: bass.Bass):
    qt_h = nc.declare_dram_parameter("qt", [NB, P, QTW], F32, isOutput=False)
    vals_h = nc.declare_dram_parameter("vals", [NB, 3, M, P, L], F16, isOutput=False)
    out_h = nc.declare_dram_parameter("out", [NB, M, P, LE], F16, isOutput=True)

    with tile.TileContext(nc) as tc, ExitStack() as ctx:
        const = ctx.enter_context(tc.tile_pool(name="const", bufs=1))
        perb = ctx.enter_context(tc.tile_pool(name="perb", bufs=2))
        sml = ctx.enter_context(tc.tile_pool(name="sml", bufs=2))
        cmp = ctx.enter_context(tc.tile_pool(name="cmp", bufs=2))
        vpool = ctx.enter_context(tc.tile_pool(name="vp", bufs=2))
        epi = ctx.enter_context(tc.tile_pool(name="epi", bufs=1))
        psg = ctx.enter_context(tc.tile_pool(name="psg", bufs=1, space="PSUM"))
        pst = ctx.enter_context(tc.tile_pool(name="pst", bufs=2, space="PSUM"))

        ident = const.tile([128, 128], F32)
        masks.make_identity(nc, ident[:])
        expb = const.tile([128, 1], F32)
        nc.vector.memset(expb[:], -23.0 * LN2)

        for b in range(NB):
            # ---- 1/nrm (Newton-refined reciprocal) ----
            nrm_t = perb.tile([128, 1], F32, tag="nrm")
            na = qt_h[b]
            nc.sync.dma_start(
                nrm_t[:],
                AP(na.tensor, na.offset + LE + L, [[0, 128], [1, 1]]))
            r0 = perb.tile([128, 1], F32, tag="r0")
            nc.vector.reciprocal(r0[:], nrm_t[:])
            er = perb.tile([128, 1], F32, tag="er")
            nc.vector.tensor_tensor(er[:], nrm_t[:], r0[:], op=OP.mult)
            nc.vector.tensor_scalar(er[:], er[:], -1.0, 2.0, op0=OP.mult, op1=OP.add)
            rr = perb.tile([128, 1], F32, tag="rr")
            nc.vector.tensor_tensor(rr[:], r0[:], er[:], op=OP.mult)

            # ---- events in [128, 128] layout: row pi = p*8+jc, col f = j%128 ----
            tt = perb.tile([128, 128], F32, tag="tt")
            ta_src = qt_h[b]
            nc.sync.dma_start(
                tt[:],
                AP(ta_src.tensor, ta_src.offset + LE,
                   [[QTW, P], [128, 8], [1, 128]]),
            )

            # ---- Veltkamp split of t and nrm ----
            ct = perb.tile([128, 128], F32, tag="ct")
            nc.vector.tensor_scalar(ct[:], tt[:], SPLIT, None, op0=OP.mult)
            th = perb.tile([128, 128], F32, tag="th")
            nc.vector.tensor_tensor(th[:], ct[:], tt[:], op=OP.subtract)  # c - t
            nc.vector.tensor_tensor(th[:], ct[:], th[:], op=OP.subtract)  # t_hi
            tl = perb.tile([128, 128], F32, tag="tl")
            nc.vector.tensor_tensor(tl[:], tt[:], th[:], op=OP.subtract)  # t_lo

            cn = perb.tile([128, 1], F32, tag="cn")
            nc.vector.tensor_scalar(cn[:], nrm_t[:], SPLIT, None, op0=OP.mult)
            nh = perb.tile([128, 1], F32, tag="nh")
            nc.vector.tensor_tensor(nh[:], cn[:], nrm_t[:], op=OP.subtract)
            nc.vector.tensor_tensor(nh[:], cn[:], nh[:], op=OP.subtract)
            nl = perb.tile([128, 1], F32, tag="nl")
            nc.vector.tensor_tensor(nl[:], nrm_t[:], nh[:], op=OP.subtract)

            # ---- P = RN(t*nrm), e1 = exact residual (Dekker) ----
            Pt = perb.tile([128, 128], F32, tag="Pt")
            nc.vector.tensor_scalar(Pt[:], tt[:], nrm_t[:], None, op0=OP.mult)
            e1 = perb.tile([128, 128], F32, tag="e1")
            nc.vector.tensor_scalar(e1[:], th[:], nh[:], None, op0=OP.mult)
            nc.vector.tensor_tensor(e1[:], e1[:], Pt[:], op=OP.subtract)
            x2 = perb.tile([128, 128], F32, tag="x2")
            nc.vector.tensor_scalar(x2[:], th[:], nl[:], None, op0=OP.mult)
            nc.vector.tensor_tensor(e1[:], e1[:], x2[:], op=OP.add)
            nc.vector.tensor_scalar(x2[:], tl[:], nh[:], None, op0=OP.mult)
            nc.vector.tensor_tensor(e1[:], e1[:], x2[:], op=OP.add)
            nc.vector.tensor_scalar(x2[:], tl[:], nl[:], None, op0=OP.mult)
            nc.vector.tensor_tensor(e1[:], e1[:], x2[:], op=OP.add)

            # ---- ulp(x) = 2^(floor(log2 x) - 23) ----
            def make_ulp(src, outtag):
                lg = perb.tile([128, 128], F32, tag="ulg")
                nc.scalar.activation(lg[:], src[:], AFT.Ln)
                nc.vector.tensor_scalar(lg[:], lg[:], 1.0 / LN2, None, op0=OP.mult)
                lgi = perb.tile([128, 128], I32, tag="ulgi")
                nc.vector.tensor_copy(lgi[:], lg[:])
                lgf = perb.tile([128, 128], F32, tag="ulgf")
                nc.vector.tensor_copy(lgf[:], lgi[:])
                fx = perb.tile([128, 128], F32, tag="ufx")
                nc.vector.tensor_tensor(fx[:], lgf[:], lg[:], op=OP.is_gt)
                nc.vector.tensor_tensor(lgf[:], lgf[:], fx[:], op=OP.subtract)
                ul = perb.tile([128, 128], F32, tag=outtag)
                nc.scalar.activation(ul[:], lgf[:], AFT.Exp, scale=LN2, bias=expb[:])
                return ul

            ulpt = make_ulp(tt, "ulpt")
            # c = ulp(t) * nrm * 0.5 ; delta = e1 + c ; w = 0.75*(1 - [delta>0])
            cterm = perb.tile([128, 128], F32, tag="cterm")
            nc.vector.tensor_scalar(cterm[:], ulpt[:], nrm_t[:], 0.5, op0=OP.mult, op1=OP.mult)
            nc.vector.tensor_tensor(cterm[:], e1[:], cterm[:], op=OP.add)
            sel = perb.tile([128, 128], F32, tag="sel")
            nc.vector.tensor_scalar(sel[:], cterm[:], 0.0, -1.0, op0=OP.is_gt, op1=OP.mult)
            nc.vector.tensor_scalar(sel[:], sel[:], 1.0, 0.75, op0=OP.add, op1=OP.mult)
            ulpP = make_ulp(Pt, "ulpP")
            nc.vector.tensor_tensor(ulpP[:], ulpP[:], sel[:], op=OP.mult)

            # ---- T = P - w*ulp(P)  in [128, 128] layout ----
            T128 = perb.tile([128, 128], F32, tag="T128")
            nc.vector.tensor_tensor(T128[:], Pt[:], ulpP[:], op=OP.subtract)
            # shifted (flat j+1) version: cols 0..126 from col+1, col 127 from
            # next partition's col 0 (BIG for the last chunk of each path)
            T128s = perb.tile([128, 128], F32, tag="T128s")
            nc.vector.tensor_copy(T128s[:, 0:127], T128[:, 1:128])
            nc.vector.memset(T128s[:, 127:128], BIG)
            for p in range(P):
                nc.sync.dma_start(T128s[8 * p:8 * p + 7, 127:128],
                                  T128[8 * p + 1:8 * p + 8, 0:1])

            # ---- transpose thresholds: TT[j%128, p*8+jc] ----
            TT = perb.tile([128, 128], F32, tag="TT")
            psT = pst.tile([128, 128], F32, tag="ps1")
            nc.tensor.matmul(psT[:], T128[:], ident[:], is_transpose=True)
            nc.scalar.activation(TT[:], psT[:], AFT.Copy)
            TTs = perb.tile([128, 128], F32, tag="TTs")
            psT2 = pst.tile([128, 128], F32, tag="ps1")
            nc.tensor.matmul(psT2[:], T128s[:], ident[:], is_transpose=True)
            nc.scalar.activation(TTs[:], psT2[:], AFT.Copy)

            # ---- exact 3-way bf16 split of t:  t = A + B + C ----
            Abf = perb.tile([128, 128], BF16, tag="Abf")
            nc.vector.tensor_copy(Abf[:], tt[:])
            Af = perb.tile([128, 128], F32, tag="Af")
            nc.vector.tensor_copy(Af[:], Abf[:])
            R1 = perb.tile([128, 128], F32, tag="R1")
            nc.vector.tensor_tensor(R1[:], tt[:], Af[:], op=OP.subtract)
            Bbf = perb.tile([128, 128], BF16, tag="Bbf")
            nc.vector.tensor_copy(Bbf[:], R1[:])
            Bf = perb.tile([128, 128], F32, tag="Bf")
            nc.vector.tensor_copy(Bf[:], Bbf[:])
            Cf = perb.tile([128, 128], F32, tag="Cf")
            nc.vector.tensor_tensor(Cf[:], R1[:], Bf[:], op=OP.subtract)

            for p in range(P):
                # ---- replicate raw q across 128 partitions ----
                qrep = sml.tile([128, LE], F32, tag="qrep")
                qa = qt_h[b, p, 0:LE]
                nc.sync.dma_start(qrep[:], AP(qa.tensor, qa.offset, [[0, 128], [1, LE]]))

                # ---- V = [mu; al; be; A; B; C] (raw values) ----
                V = vpool.tile([NV, L], F32, tag="V")
                V16 = vpool.tile([96, L], F16, tag="V16")
                nc.sync.dma_start(V16[0:32, :], vals_h[b, 0, :, p, :])
                nc.sync.dma_start(V16[32:64, :], vals_h[b, 1, :, p, :])
                nc.sync.dma_start(V16[64:96, :], vals_h[b, 2, :, p, :])
                nc.vector.tensor_copy(V[0:96, :], V16[0:96, :])
                nc.sync.dma_start(V[96:97, :], Af[p * 8:(p + 1) * 8, :])
                nc.sync.dma_start(V[97:98, :], Bf[p * 8:(p + 1) * 8, :])
                nc.sync.dma_start(V[98:99, :], Cf[p * 8:(p + 1) * 8, :])

                VT = vpool.tile([128, NV * 8], F32R, tag="VT")
                for jc in range(8):
                    psv = pst.tile([128, NV], F32, tag="ps1")
                    nc.tensor.matmul(psv[:], V[:, jc * 128:(jc + 1) * 128],
                                     ident[0:NV, 0:NV], is_transpose=True)
                    nc.scalar.activation(VT[:, jc * NV:(jc + 1) * NV], psv[:], AFT.Copy)

                # ---- compares + one-hot + gather matmuls ----
                G = psg.tile([NV, LE], F32, tag="G")
                mask_f = epi.tile([1, LE], F32, tag="mask")
                for c in range(8):
                    Sc = cmp.tile([128, LE], BF16, tag="Sc")
                    nc.vector.tensor_tensor(
                        Sc[:], qrep[:], _bcast(TT[:, p * 8 + c:p * 8 + c + 1], LE),
                        op=OP.is_gt)
                    S2c = cmp.tile([128, LE], BF16, tag="S2c")
                    nc.vector.tensor_tensor(
                        S2c[:], qrep[:], _bcast(TTs[:, p * 8 + c:p * 8 + c + 1], LE),
                        op=OP.is_gt)
                    oh = cmp.tile([128, LE], F32R, tag="oh")
                    nc.vector.tensor_tensor(oh[:], Sc[:], S2c[:], op=OP.subtract)
                    if c == 0:
                        nc.vector.tensor_scalar(oh[0:1, :], S2c[0:1, :], -1.0, 1.0,
                                                op0=OP.mult, op1=OP.add)
                        nc.vector.tensor_copy(mask_f[:], Sc[0:1, :])
                    for f in range(4):
                        nc.tensor.matmul(
                            G[:, f * 512:(f + 1) * 512],
                            VT[:, c * NV:(c + 1) * NV],
                            oh[:, f * 512:(f + 1) * 512],
                            start=(c == 0), stop=(c == 7))

                # ---- epilogue ----
                Gmu = epi.tile([32, LE], F32, tag="Gmu")
                nc.scalar.activation(Gmu[:], G[0:32, :], AFT.Copy, scale=rr[0:32])
                Gal = epi.tile([32, LE], F32, tag="Gal")
                nc.scalar.activation(Gal[:], G[32:64, :], AFT.Copy, scale=rr[32:64])
                Gbe = epi.tile([32, LE], F32, tag="Gbe")
                nc.scalar.activation(Gbe[:], G[64:96, :], AFT.Copy, scale=-1.0)
                Gt = epi.tile([3, LE], F32, tag="Gt")
                nc.scalar.activation(Gt[:], G[96:99, :], AFT.Copy)
                Gt1 = epi.tile([1, LE], F32, tag="Gt1")
                nc.sync.dma_start(Gt1[:], Gt[1:2, :])
                Gt2 = epi.tile([1, LE], F32, tag="Gt2")
                nc.sync.dma_start(Gt2[:], Gt[2:3, :])

                tsum = epi.tile([1, LE], F32, tag="tsum")
                nc.vector.tensor_tensor(tsum[:], Gt[0:1, :], Gt1[:], op=OP.add)
                nc.vector.tensor_tensor(tsum[:], tsum[:], Gt2[:], op=OP.add)
                nc.vector.tensor_tensor(tsum[:], tsum[:], mask_f[:], op=OP.mult)

                qrow = epi.tile([1, LE], F32, tag="qrow")
                nc.sync.dma_start(qrow[:], qt_h[b, p, 0:LE])
                dtt = epi.tile([32, LE], F32, tag="dtt")
                nc.vector.tensor_scalar(qrow[:], qrow[:], rr[0:1], None, op0=OP.mult)
                nc.vector.tensor_tensor(dtt[0:1, :], qrow[:], tsum[:], op=OP.subtract)
                dtrep = epi.tile([32, LE], F32, tag="dtrep")
                nc.vector.stream_shuffle(dtrep[:], dtt[:], mask=[0] * 32)

                u = epi.tile([32, LE], F32, tag="u")
                nc.vector.tensor_tensor(u[:], Gbe[:], dtrep[:], op=OP.mult)
                E = epi.tile([32, LE], F32, tag="E")
                nc.scalar.activation(E[:], u[:], AFT.Exp)
                O = epi.tile([32, LE], F32, tag="O")
                nc.vector.tensor_tensor(O[:], Gal[:], Gmu[:], op=OP.subtract)
                nc.vector.tensor_tensor(O[:], O[:], E[:], op=OP.mult)
                O16 = epi.tile([32, LE], F16, tag="O16")
                nc.vector.tensor_tensor(O16[:], O[:], Gmu[:], op=OP.add)
                nc.sync.dma_start(out_h[b, :, p, :], O16[:])

    nc.compile()
    return nc


def _host_globals(query_times, event_times, mu, alpha, beta, norm_constants):
    """Global (all-cores) input arrays, already in the sharded layout: axis 0
    is the batch dim that shard_map splits 8 ways (2 batches per core)."""
    q = np.ascontiguousarray(query_times, dtype=np.float32)
    t = np.ascontiguousarray(event_times, dtype=np.float32)
    nrm = np.ascontiguousarray(norm_constants, dtype=np.float32)
    qtn = np.empty((B, P, QTW), np.float32)
    qtn[:, :, 0:LE] = q
    qtn[:, :, LE:LE + L] = t
    qtn[:, :, LE + L:] = nrm[:, None, None]
    vals = np.empty((B, 3, M, P, L), np.float16)
    vals[:, 0] = mu
    vals[:, 1] = alpha
    vals[:, 2] = beta
    return {"qt": qtn, "vals": vals}


def _per_core_maps(g):
    return [
        {k: np.ascontiguousarray(v[NB * i:NB * (i + 1)]) for k, v in g.items()}
        for i in range(NCORES)
    ]


_NC_CACHE = {}
_SETUP: dict = {}


def _aot_cache_path():
    import hashlib
    h = hashlib.sha256(open(__file__, "rb").read()).hexdigest()[:16]
    import os
    d = os.path.expanduser("~/.neuron-compile-cache")
    os.makedirs(d, exist_ok=True)
    return os.path.join(d, f"hawkes_aot_{h}.pkl")


def _try_load_aot():
    """Load serialized AOT executables from disk; skips build + compile."""
    import pickle
    import jax
    from jax.experimental.serialize_executable import deserialize_and_load
    from jax.sharding import Mesh, NamedSharding, PartitionSpec

    with open(_aot_cache_path(), "rb") as f:
        blob = pickle.load(f)
    devices = jax.devices()[:NCORES]
    assert len(devices) == NCORES
    mesh = Mesh(np.asarray(devices), ("core",))
    ns = NamedSharding(mesh, PartitionSpec("core"))
    compiled = deserialize_and_load(*blob["compiled"])
    zero_fn = deserialize_and_load(*blob["zeros"])
    _SETUP.update(
        groups=[dict(ns=ns, compiled=compiled, zero_fn=zero_fn, ncore=NCORES)],
        in_names=blob["in_names"], out_names=blob["out_names"], jax=jax,
    )
    return _SETUP


def _save_aot():
    import pickle
    from jax.experimental.serialize_executable import serialize

    g = _SETUP["groups"][0]
    blob = {
        "compiled": serialize(g["compiled"]),
        "zeros": serialize(g["zero_fn"]),
        "in_names": _SETUP["in_names"],
        "out_names": _SETUP["out_names"],
    }
    tmp = _aot_cache_path() + ".tmp"
    with open(tmp, "wb") as f:
        pickle.dump(blob, f)
    import os
    os.replace(tmp, _aot_cache_path())


def _ensure_setup():
    """One-time: build the Bass program, AOT-compile the sharded PJRT
    executable (NEFF comes from the persistent neuron cache when warm), and
    prepare a device-side zero-buffer maker for the donated outputs.  Doing
    this at import keeps the kernel() call itself transfer-bound."""
    if "groups" in _SETUP:
        return _SETUP
    try:
        return _try_load_aot()
    except Exception:
        pass
    import jax
    import jax.numpy as jnp
    from jax.experimental.shard_map import shard_map
    from jax.sharding import Mesh, NamedSharding, PartitionSpec
    from concourse import bass2jax as B2J

    nc = _NC_CACHE.get("nc")
    if nc is None:
        nc = _NC_CACHE["nc"] = build_program(bacc.Bacc())
    B2J.install_neuronx_cc_hook()

    partition_name = nc.partition_id_tensor.name if nc.partition_id_tensor else None
    in_names, in_avals, out_names, out_avals = [], [], [], []
    for alloc in nc.m.functions[0].allocations:
        if not isinstance(alloc, mybir.MemoryLocationSet):
            continue
        name = alloc.memorylocations[0].name
        if alloc.kind == "ExternalInput":
            if name != partition_name:
                in_names.append(name)
                in_avals.append((tuple(alloc.tensor_shape), mybir.dt.np(alloc.dtype)))
        elif alloc.kind == "ExternalOutput":
            out_names.append(name)
            out_avals.append(jax.core.ShapedArray(tuple(alloc.tensor_shape),
                                                  mybir.dt.np(alloc.dtype)))
    n_params, n_outs = len(in_names), len(out_avals)
    all_names = list(in_names) + list(out_names)
    if partition_name is not None:
        all_names.append(partition_name)
    donate = tuple(range(n_params, n_params + n_outs))

    def _body(*args):
        operands = list(args)
        if partition_name is not None:
            operands.append(B2J.partition_id_tensor())
        outs = B2J._bass_exec_p.bind(
            *operands,
            out_avals=tuple(out_avals),
            in_names=tuple(all_names),
            out_names=tuple(out_names),
            lowering_input_output_aliases=(),
            sim_require_finite=True,
            sim_require_nnan=True,
            nc=nc,
        )
        return tuple(outs)

    all_devices = jax.devices()[:NCORES]
    assert len(all_devices) == NCORES
    spec = PartitionSpec("core")

    def make_group(devices):
        ncore = len(devices)
        mesh = Mesh(np.asarray(devices), ("core",))
        ns = NamedSharding(mesh, spec)
        sharded = jax.jit(
            shard_map(_body, mesh=mesh, in_specs=(spec,) * (n_params + n_outs),
                      out_specs=(spec,) * n_outs, check_rep=False),
            donate_argnums=donate, keep_unused=True,
        )
        in_structs = [
            jax.ShapeDtypeStruct((ncore * s[0], *s[1:]), d, sharding=ns)
            for (s, d) in in_avals
        ]
        out_structs = [
            jax.ShapeDtypeStruct((ncore * a.shape[0], *a.shape[1:]), a.dtype,
                                 sharding=ns)
            for a in out_avals
        ]
        compiled = sharded.lower(*in_structs, *out_structs).compile()
        zero_specs = [((ncore * a.shape[0], *a.shape[1:]), a.dtype)
                      for a in out_avals]
        zero_fn = jax.jit(
            lambda: tuple(jnp.zeros(s, d) for (s, d) in zero_specs),
            out_shardings=(ns,) * n_outs,
        ).lower().compile()
        return dict(ns=ns, compiled=compiled, zero_fn=zero_fn, ncore=ncore)

    # Single full-machine group: split-pipelining measured slower on the
    # axon relay (head-of-line blocking between interleaved up/down streams).
    groups = [make_group(all_devices)]

    _SETUP.update(
        groups=groups, in_names=in_names, out_names=out_names,
        out_avals=out_avals, jax=jax,
    )
    try:
        _save_aot()
    except Exception:
        pass
    return _SETUP


def _put_sharded(jax, arr, ns):
    """Per-shard async device_put + global assembly (overlaps host work of
    later shards with relay transfer of earlier ones)."""
    devs = list(ns.mesh.devices.flat)
    n = len(devs)
    step = arr.shape[0] // n
    shards = [
        jax.device_put(arr[i * step:(i + 1) * step], devs[i]) for i in range(n)
    ]
    return jax.make_array_from_single_device_arrays(arr.shape, ns, shards)


def _run_fast(query_times, event_times, mu, alpha, beta, norm_constants):
    s = _ensure_setup()
    jax = s["jax"]
    g = s["groups"][0]
    ns = g["ns"]
    devs = list(ns.mesh.devices.flat)

    zeros = g["zero_fn"]()  # device-side, dispatched before uploads begin

    q = np.ascontiguousarray(query_times, dtype=np.float32)
    t = np.ascontiguousarray(event_times, dtype=np.float32)
    nrm = np.ascontiguousarray(norm_constants, dtype=np.float32)
    qtn = np.empty((B, P, QTW), np.float32)
    qtn[:, :, 0:LE] = q
    qtn[:, :, LE:LE + L] = t
    qtn[:, :, LE + L:] = nrm[:, None, None]
    qt_g = _put_sharded(jax, qtn, ns)

    # vals: cast each core's shard to fp16 then queue its upload immediately,
    # so the next shard's cast overlaps the previous shard's relay transfer.
    vals_shards = []
    for i in range(NCORES):
        sl = slice(NB * i, NB * (i + 1))
        ch = np.empty((NB, 3, M, P, L), np.float16)
        ch[:, 0] = mu[sl]
        ch[:, 1] = alpha[sl]
        ch[:, 2] = beta[sl]
        vals_shards.append(jax.device_put(ch, devs[i]))
    vals_g = jax.make_array_from_single_device_arrays(
        (B, 3, M, P, L), ns, vals_shards)
    by_name = {"qt": qt_g, "vals": vals_g}
    dev_in = [by_name[name] for name in s["in_names"]]
    outs = g["compiled"](*dev_in, *zeros)

    out_g = outs[s["out_names"].index("out")]
    res = np.empty((B, M, P, LE), np.float32)
    import concurrent.futures as cf

    def fetch(sh):
        res[sh.index] = np.asarray(sh.data)  # 4MB fetch + fp16->f32 cast

    with cf.ThreadPoolExecutor(max_workers=NCORES) as ex:
        list(ex.map(fetch, out_g.addressable_shards))
    return res


def _numpy_fallback(query_times, event_times, mu, alpha, beta, norm_constants):
    q_norm = (query_times / norm_constants[:, None, None]).astype(np.float32)
    Bq, Pq, Le = q_norm.shape
    last = np.empty((Bq, Pq, Le), np.int64)
    for b in range(Bq):
        for p in range(Pq):
            last[b, p] = np.searchsorted(event_times[b, p], q_norm[b, p], "left") - 1
    idx = np.clip(last, 0, None)
    g = np.broadcast_to(idx[:, None], (Bq, mu.shape[1], Pq, Le))
    mu_l = np.take_along_axis(mu, g, 3)
    al_l = np.take_along_axis(alpha, g, 3)
    be_l = np.take_along_axis(beta, g, 3)
    t_l = np.where(last == -1, 0.0, np.take_along_axis(event_times, idx, 2))
    dt = (q_norm - t_l)[:, None]
    out = mu_l + (al_l - mu_l) * np.exp(-be_l * dt)
    return (out / norm_constants[:, None, None, None]).astype(np.float32)


def kernel(query_times, event_times, mu, alpha, beta, norm_constants):
    try:
        return _run_fast(query_times, event_times, mu, alpha, beta,
                         norm_constants)
    except Exception as e:
        print(f"WARNING: fast path failed ({e!r}); trying run_bass_kernel_spmd",
              file=sys.stderr)
    g = _host_globals(query_times, event_times, mu, alpha, beta, norm_constants)
    try:
        from concourse.bass_utils import run_bass_kernel_spmd

        if "nc" not in _NC_CACHE:
            _NC_CACHE["nc"] = build_program(bacc.Bacc())
        nc = _NC_CACHE["nc"]
        res = run_bass_kernel_spmd(nc, _per_core_maps(g), core_ids=list(range(NCORES)))
        outs = [res.results[i]["out"].astype(np.float32) for i in range(NCORES)]
        return np.concatenate(outs, axis=0)
    except Exception as e:  # pragma: no cover
        print(
            "WARNING: device execution failed; returning HOST numpy fallback "
            f"(no hardware time was measured). Device error: {e!r}",
            file=sys.stderr,
        )
        return _numpy_fallback(
            np.asarray(query_times, np.float32),
            np.asarray(event_times, np.float32),
            np.asarray(mu, np.float32),
            np.asarray(alpha, np.float32),
            np.asarray(beta, np.float32),
            np.asarray(norm_constants, np.float32),
        )


# Import-time warmup: build + AOT compile so the kernel() call itself is
# transfer-bound.  Best-effort — kernel() retries lazily on failure.
try:
    _ensure_setup()
except Exception:
    pass
